# revision 8
# baseline (speedup 1.0000x reference)
"""Trainium2 Bass kernel for nn_Net_21852793602541 (gnn_message_passing).

The reference net's output depends only on a tiny dependency cone of the
message-passing graph: the final hidden layer reads the wave-2 snapshot of
neuron activations, so only neurons feeding neuron 255 through channels whose
source was already processed matter.  For the fixed graph that is a 3-conv
chain (x -> n0 -> n172 -> n215), one 784->200 FC block, a 200->10 FC and
log_softmax.  The cone is recomputed at runtime from the src/tgt inputs.

Per-core mapping (data-parallel over batch, 16 images/core on 8 cores):
  * 5x5 conv == one PE accumulation group: contraction K = (dy, slot-row)
    (5 row-offsets x 32 padded columns = 160 -> matmuls of K=128/32) with a
    banded-Toeplitz stationary (fp16) against 5 y-shifted slot copies of the
    padded image block (fp16), N = (y, b) = 448.  Slot free layout is
    (y, b) so every slot group is one contiguous span: the input image is
    DMA-replicated into all 5 groups straight from DRAM (no on-chip copies),
    and each conv's relu+bias lands in the downstream slot groups as five
    direct PSUM->SBUF writes split across the Scalar and Vector engines.
  * fc1 streams the 200 hidden units as the moving operand (7 accumulated
    matmuls, stationary = 16-wide activation slices); the fc1 bias rides a
    spare zero row of the activation stack (set to 1) paired with a bias row
    in the weight block.
  * fc2 is computed transposed (stationary = hidden activations) so logits
    land directly as [batch, cls]; its bias rides an extra ones-row of h2.
  * log_softmax without max-subtraction (logits are bounded); ln(sum) via
    the float-exponent bitcast approximation (err < 0.03 nats, tolerance is
    2e-2 relative), which avoids the 1.3us Ln ACT-table load entirely.
"""

import numpy as np

import concourse.bass as bass
import concourse.tile as tile
from concourse import bacc, mybir
from concourse.bass_utils import run_bass_kernel_spmd

# The axon NTFF profile hook normally lives in antenv.axon_hooks, which this
# image lacks.  Shim it from the boot module's ctypes implementation so
# BASS_TRACE=1 profiling works; degrade silently if unavailable.
try:
    import antenv.axon_hooks  # noqa: F401
except ImportError:
    try:
        import sys as _sys
        import types as _types

        from trn_agent_boot.trn_boot import _ntff_profile_via_ctypes

        _hook = _ntff_profile_via_ctypes('/opt/axon/libaxon_pjrt.so')
        _mod = _types.ModuleType('antenv.axon_hooks')
        _mod.get_axon_ntff_profile_hook = lambda: _hook
        _mod.set_axon_ntff_profile_hook = lambda h: None
        _sys.modules['antenv.axon_hooks'] = _mod
    except Exception:
        pass

F32 = mybir.dt.float32
F16 = mybir.dt.float16
I32 = mybir.dt.int32
AF = mybir.ActivationFunctionType
ALU = mybir.AluOpType
N_NEURONS = 256
N_CORES = 8
B_TOTAL = 128
B = B_TOTAL // N_CORES  # 16 images per core
HW = 28
FC_HID = 200
N_CLS = 10

# float-exponent log: ln(x) ~ ln2 * (bits(x) * 2^-23 - 126.94269504089732)
LOG_SCALE = -float(np.log(2.0)) / (1 << 23)
LOG_BIAS = 126.94269504089732 * float(np.log(2.0))

LAST_RESULT = None  # BassKernelResults of the most recent run (for profiling)


# ---------------------------------------------------------------- schedule
def _schedule(src, tgt):
    n = N_NEURONS
    in_lists = [src[np.where(tgt == i)[0]].astype(np.int64).tolist() for i in range(n)]
    waves = []
    processed = np.zeros(n, bool)
    frontier = [0]
    while True:
        waves.append(list(frontier))
        processed[frontier] = True
        if processed[n - 1]:
            break
        nxt = set()
        for v in frontier:
            for m in tgt[src == v]:
                if not processed[m]:
                    nxt.add(int(m))
        frontier = sorted(nxt)
        assert frontier, "last neuron unreachable"
    return in_lists, waves


def _cone(src, tgt):
    """Returns (steps, fc_live).

    steps: ordered list of (node, [(srckey, channel), ...]) where srckey is
      'x' for the image input or an int neuron id computed in an earlier step.
    fc_live: [(channel_of_255, src_node), ...] live channels of the readout.
    """
    n = N_NEURONS
    in_lists, waves = _schedule(src, tgt)
    wave_of = {}
    for wi, w in enumerate(waves):
        for v in w:
            if v not in wave_of:
                wave_of[v] = wi
    BIG = 1 << 30
    w255 = wave_of[n - 1]
    fc_live = [(c, int(s)) for c, s in enumerate(in_lists[n - 1])
               if wave_of.get(int(s), BIG) < w255]

    live = {}
    stack = [s for _, s in fc_live]
    seen = set()
    while stack:
        v = stack.pop()
        if v in seen:
            continue
        seen.add(v)
        if v == 0:
            live[0] = [('x', 0)]
            continue
        chans = [(int(s), c) for c, s in enumerate(in_lists[v])
                 if wave_of.get(int(s), BIG) < wave_of[v]]
        assert chans, f"cone node {v} has no live channels"
        live[v] = [(s, c) for s, c in chans]
        stack += [s for s, _ in chans]

    steps = sorted(live.items(), key=lambda kv: wave_of[kv[0]])
    return steps, fc_live


# ---------------------------------------------------------- host-side packing
def _toeplitz(w):
    """w [5,5] -> [160, 28] banded matrix over K=(dy, row).

    Slot row r of each 32-row group holds padded-image column (r+2) mod 32,
    so the activation value at x lands at row x (32-aligned writes; wrapped
    rows 28..31 hold the zero x-padding)."""
    T = np.zeros((160, HW), np.float32)
    for dy in range(5):
        for dx in range(5):
            for xc in range(HW):
                T[dy * 32 + (xc + dx - 2) % 32, xc] = w[dy, dx]
    return T


class _Layout:
    def __init__(self):
        self.n = 0

    def alloc(self, w):
        c0 = self.n
        self.n += w
        return c0


def _pack(steps, fc_live, conv_w, conv_b, fc1_w, fc1_b, fc2_w, fc2_b):
    """Builds consts (f32), mainh toeplitz block (fp16), f1w (fp16)."""
    slots = {}
    lay32 = _Layout()
    lay16 = _Layout()
    for v, chans in steps:
        for j, _ in enumerate(chans):
            slots[('toepA', v, j)] = lay16.alloc(HW)
            slots[('toepB', v, j)] = lay16.alloc(HW)
        slots[('cb', v)] = lay32.alloc(1)
    slots['ident'] = lay32.alloc(B)
    slots['fc2wA'] = lay32.alloc(N_CLS)
    slots['fc2wB'] = lay32.alloc(N_CLS)
    slots['fc1bT'] = lay32.alloc(FC_HID)

    C = np.zeros((128, lay32.n), np.float32)
    TH = np.zeros((128, lay16.n), np.float16)
    for v, chans in steps:
        for j, (skey, ch) in enumerate(chans):
            T = _toeplitz(conv_w[v, 0, ch])
            TH[:, slots[('toepA', v, j)]:slots[('toepA', v, j)] + HW] = T[:128]
            TH[:32, slots[('toepB', v, j)]:slots[('toepB', v, j)] + HW] = T[128:]
        C[:HW, slots[('cb', v)]] = conv_b[v]
    C[:B, slots['ident']:slots['ident'] + B] = np.eye(B, dtype=np.float32)
    w2t = fc2_w.T  # [200, 10]
    C[:128, slots['fc2wA']:slots['fc2wA'] + N_CLS] = w2t[:128]
    C[:FC_HID - 128, slots['fc2wB']:slots['fc2wB'] + N_CLS] = w2t[128:]
    C[FC_HID - 128, slots['fc2wB']:slots['fc2wB'] + N_CLS] = fc2_b  # ht ones col
    C[:B, slots['fc1bT']:slots['fc1bT'] + FC_HID] = fc1_b[None, :]

    n_fc = len(fc_live)
    f1p = np.zeros((128, 1400 * n_fc), np.float16)
    for k, (c, s) in enumerate(fc_live):
        blk = fc1_w[:, c * 784:(c + 1) * 784].reshape(FC_HID, HW, HW)  # [h, y, x]
        arr = blk.reshape(FC_HID, 4, 7, HW).transpose(1, 3, 2, 0)  # [yg, x, ysub, h]
        f1p[:, k * 1400:(k + 1) * 1400] = np.pad(
            arr, ((0, 0), (0, 4), (0, 0), (0, 0))).reshape(128, 1400)
    return C, TH, f1p, slots


# ---------------------------------------------------------- device program
def _build(steps, fc_live, ncols32, ncols16, nfc):
    nc = bacc.Bacc("TRN2", target_bir_lowering=False)
    consts_d = nc.dram_tensor("consts", [128, ncols32], F32, kind="ExternalInput")
    mainh_d = nc.dram_tensor("mainh", [128, ncols16], F16, kind="ExternalInput")
    f1w_d = nc.dram_tensor("f1w", [128, 1400 * nfc], F16, kind="ExternalInput")
    xin_d = nc.dram_tensor("xin", [HW, HW * B], F16, kind="ExternalInput")
    out_d = nc.dram_tensor("out", [B, N_CLS], F32, kind="ExternalOutput")

    feeds_conv = set()
    for v, chans in steps:
        for skey, _ in chans:
            if skey != 'x':
                feeds_conv.add(skey)
    fc_srcs = [s for _, s in fc_live]
    SL = _SLOTS

    with tile.TileContext(nc) as tc:
        with (
            tc.tile_pool(name="persist", bufs=1) as pool,
            tc.tile_pool(name="cpsum", bufs=1, space="PSUM") as cpp,
            tc.tile_pool(name="fpsum", bufs=1, space="PSUM") as fpp,
        ):
            consts = pool.tile([128, ncols32], F32, tag="consts")
            mainh = pool.tile([128, ncols16], F16, tag="mainh")
            f1w = pool.tile([128, 1400 * nfc], F16, tag="f1w")

            # preload the Exp activation table during the input-DMA shadow
            # (the tail's exp would otherwise eat a 1.3us table load)
            swu = pool.tile([1, 2], F32, tag="swu")
            nc.scalar.memzero(swu[:])
            nc.scalar.activation(swu[:, 0:1], swu[:, 0:1], AF.Exp)

            # activation slot tiles (x + each conv producer); free layout is
            # (y, b) so each 32-row dy group is one contiguous 448-elem span
            stacks = {}
            for key in ['x'] + sorted(feeds_conv):
                a = pool.tile([128, 32 * B], F16, name=f"stA_{key}", tag=f"stA_{key}")
                b = pool.tile([32, 32 * B], F16, name=f"stB_{key}", tag=f"stB_{key}")
                eng = nc.vector if key == 'x' else nc.gpsimd
                eng.memset(a[:], 0.0)
                eng.memset(b[:], 0.0)
                stacks[key] = (a, b)
            fcstacks = {}
            for sv in set(fc_srcs):
                t = pool.tile([128, 7 * B], F16, name=f"fcst_{sv}", tag=f"fcst_{sv}")
                nc.scalar.memzero(t[:])
                fcstacks[sv] = t
            # ht's extra ones column rides through the transpose into h2's
            # last row and carries the fc2 bias through the fc2 matmul
            ht = pool.tile([B, FC_HID + 1], F32, tag="ht")
            nc.vector.memset(ht[:, FC_HID:FC_HID + 1], 1.0)

            # input DMAs: x replicated into four of the five y-shifted slot
            # groups straight from DRAM (slot group dy holds image row y at
            # column span (4-dy)*B ..); only sync/scalar/gpsimd can issue
            # DMAs, so the dy=0 group is a Vector copy of the dy=1 group
            xa, xb = stacks['x']
            nc.sync.dma_start(mainh[:], mainh_d[:])
            nc.sync.dma_start(xa[96:96 + HW, 1 * B:(1 + HW) * B], xin_d[:])
            nc.scalar.dma_start(xa[64:64 + HW, 2 * B:(2 + HW) * B], xin_d[:])
            nc.scalar.dma_start(xa[32:32 + HW, 3 * B:(3 + HW) * B], xin_d[:])
            nc.scalar.dma_start(consts[:], consts_d[:])
            nc.gpsimd.dma_start(xb[0:HW, 0:HW * B], xin_d[:])
            fh = (1400 * nfc) // 2
            nc.gpsimd.dma_start(f1w[:, :fh], f1w_d[:, :fh])
            nc.gpsimd.dma_start(f1w[:, fh:], f1w_d[:, fh:])
            nc.vector.tensor_copy(xa[0:HW, 4 * B:(4 + HW) * B],
                                  xa[32:32 + HW, 3 * B:(3 + HW) * B])

            # --- conv chain ---
            for v, chans in steps:
                cb0 = SL[('cb', v)]
                bias = consts[:HW, cb0:cb0 + 1]
                nch = len(chans)

                ps = cpp.tile([HW, HW * B], F32, tag="convps")
                for j, (skey, ch) in enumerate(chans):
                    a0 = SL[('toepA', v, j)]
                    b0 = SL[('toepB', v, j)]
                    a, b = stacks[skey]
                    nc.tensor.matmul(ps[:], mainh[:, a0:a0 + HW],
                                     a[:, 2 * B:(2 + HW) * B],
                                     start=(j == 0), stop=False)
                    nc.tensor.matmul(ps[:], mainh[:32, b0:b0 + HW],
                                     b[:, 2 * B:(2 + HW) * B],
                                     start=False, stop=(j == nch - 1))

                if v in feeds_conv:
                    # relu+bias lands directly in the five slot groups of this
                    # node's stack: 2 writes on Scalar, 3 on Vector, no copies
                    a, b = stacks[v]
                    dsts = [a[dy * 32:dy * 32 + HW, (4 - dy) * B:(32 - dy) * B]
                            for dy in range(4)] + [b[0:HW, 0:HW * B]]
                    for i, dst in enumerate(dsts):
                        if i < 2:
                            nc.scalar.activation(dst, ps[:], AF.Relu,
                                                 bias=bias, scale=1.0)
                        else:
                            nc.vector.tensor_scalar(dst, ps[:], bias, 0.0,
                                                    ALU.add, ALU.max)
                if v in fcstacks:
                    # reshape y into (ygroup, ysub) partition quarters:
                    # 4 PSUM->SBUF relu writes split across Scalar/Vector
                    fst = fcstacks[v]
                    for g in range(4):
                        dst = fst[g * 32:g * 32 + HW, :]
                        srcp = ps[:, 7 * g * B:(7 * g + 7) * B]
                        if g % 2 == 0:
                            nc.scalar.activation(dst, srcp, AF.Relu,
                                                 bias=bias, scale=1.0)
                        else:
                            nc.vector.tensor_scalar(dst, srcp, bias, 0.0,
                                                    ALU.add, ALU.max)

            # --- fc1: activations stationary, hidden units streamed ---
            p1 = fpp.tile([B, FC_HID], F32, tag="p1")
            for k in range(nfc):
                fst = fcstacks[fc_live[k][1]]
                for sj in range(7):
                    i = k * 7 + sj
                    nc.tensor.matmul(p1[:], fst[:, sj * B:(sj + 1) * B],
                                     f1w[:, i * 200:(i + 1) * 200],
                                     start=(i == 0), stop=(i == 7 * nfc - 1))
            # add fc1 bias (broadcast rows in consts); relu happens after the
            # transpose, fused into the PSUM->SBUF moves
            nc.vector.tensor_add(ht[:, 0:FC_HID], p1[:],
                                 consts[:B, SL['fc1bT']:SL['fc1bT'] + FC_HID])
            idn = consts[:B, SL['ident']:SL['ident'] + B]
            t1 = fpp.tile([128, B], F32, tag="tt", bufs=2)
            t2 = fpp.tile([FC_HID - 128 + 1, B], F32, tag="tt", bufs=2)
            nc.tensor.transpose(t1[:], ht[:, 0:128], idn)
            nc.tensor.transpose(t2[:], ht[:, 128:FC_HID + 1], idn)
            h1 = pool.tile([128, B], F32, tag="h1")
            h2 = pool.tile([FC_HID - 128 + 1, B], F32, tag="h2")
            nc.scalar.activation(h1[:], t1[:], AF.Relu, bias=0.0, scale=1.0)
            nc.vector.tensor_scalar_max(h2[:], t2[:], 0.0)

            # --- fc2 transposed (stationary = hidden) + log_softmax ---
            pst = fpp.tile([B, N_CLS], F32, tag="l2", bufs=1)
            nc.tensor.matmul(pst[:], h1[:],
                             consts[:128, SL['fc2wA']:SL['fc2wA'] + N_CLS],
                             start=True, stop=False)
            nc.tensor.matmul(pst[:], h2[:],
                             consts[:FC_HID - 128 + 1, SL['fc2wB']:SL['fc2wB'] + N_CLS],
                             start=False, stop=True)
            # log_softmax without max-subtraction (logits bounded, fp32 exp
            # safe); lse via float-exponent bitcast, so no Ln table load
            ex = pool.tile([B, N_CLS], F32, tag="ex")
            sm = pool.tile([B, 1], F32, tag="sm")
            nc.scalar.activation(ex[:], pst[:], AF.Exp, bias=0.0, scale=1.0,
                                 accum_out=sm[:])
            kf = pool.tile([B, 1], F32, tag="kf")
            nc.vector.tensor_copy(kf[:], sm[:].bitcast(I32))
            nls = pool.tile([B, 1], F32, tag="nls")
            nc.vector.tensor_scalar(nls[:], kf[:], LOG_SCALE, LOG_BIAS,
                                    ALU.mult, ALU.add)
            res = pool.tile([B, N_CLS], F32, tag="res")
            nc.scalar.activation(res[:], pst[:], AF.Identity, bias=nls[:], scale=1.0)
            nc.sync.dma_start(out_d[:], res[:])
    nc.compile()
    return nc


_SLOTS = None
_PROG_CACHE = {}


def kernel(x, src, tgt, conv_w, conv_b, fc1_w, fc1_b, fc2_w, fc2_b):
    global _SLOTS, LAST_RESULT
    x = np.asarray(x, np.float32)
    src = np.asarray(src, np.int32)
    tgt = np.asarray(tgt, np.int32)
    conv_w = np.asarray(conv_w, np.float32)
    conv_b = np.asarray(conv_b, np.float32)
    fc1_w = np.asarray(fc1_w, np.float32)
    fc1_b = np.asarray(fc1_b, np.float32)
    fc2_w = np.asarray(fc2_w, np.float32)
    fc2_b = np.asarray(fc2_b, np.float32)

    steps, fc_live = _cone(src, tgt)
    C, TH, f1p, slots = _pack(steps, fc_live, conv_w, conv_b,
                              fc1_w, fc1_b, fc2_w, fc2_b)
    _SLOTS = slots

    key = (tuple((v, tuple(ch)) for v, ch in steps), tuple(fc_live),
           C.shape[1], TH.shape[1])
    if key not in _PROG_CACHE:
        _PROG_CACHE[key] = _build(steps, fc_live, C.shape[1], TH.shape[1],
                                  len(fc_live))
    nc = _PROG_CACHE[key]

    xs = x[:, 0]  # [128, 28, 28]
    in_maps = []
    for c in range(N_CORES):
        xst = xs[c * B:(c + 1) * B].transpose(2, 1, 0)  # [x, y, b]
        xst = np.ascontiguousarray(xst).reshape(HW, HW * B).astype(np.float16)
        in_maps.append({"consts": C, "mainh": TH, "f1w": f1p, "xin": xst})

    LAST_RESULT = run_bass_kernel_spmd(nc, in_maps, list(range(N_CORES)))
    out = np.concatenate([r["out"] for r in LAST_RESULT.results], axis=0)
    return out.astype(np.float32)


# revision 10
# speedup vs baseline: 1.1223x; 1.1223x over previous
"""Trainium2 Bass kernel for nn_Net_21852793602541 (gnn_message_passing).

The reference net's output depends only on a tiny dependency cone of the
message-passing graph: the final hidden layer reads the wave-2 snapshot of
neuron activations, so only neurons feeding neuron 255 through channels whose
source was already processed matter.  For the fixed graph that is a 3-conv
chain (x -> n0 -> n172 -> n215), one 784->200 FC block, a 200->10 FC and
log_softmax.  The cone is recomputed at runtime from the src/tgt inputs.

Per-core mapping (data-parallel over batch, 16 images/core on 8 cores):
  * 5x5 conv == one PE accumulation group: contraction K = (dy, slot-row)
    over four y-shifted slot groups (fp16, free layout (y, b) so each group
    is a contiguous span) against a banded-Toeplitz stationary (fp16),
    N = (y, b) = 448.  The fifth (dy=4) tap group needs no storage: since
    every group holds the same data shifted one column per dy, its matmul
    reads the dy=3 group at a +1-column window (quadrant-aligned offset 96).
    The dy=4 matmul runs FIRST in the accumulation group so it overlaps the
    Vector slot-replication copies that the dy0/dy1 groups still need.
  * Each conv's relu+bias is one Scalar PSUM->SBUF pass into the dy=2 slot
    group; Vector replicates it to dy3/dy1/dy0 (in that order, so the next
    conv's dy=4 matmul can start after the first copy).
  * fc1 streams the 200 hidden units as the moving operand (7 accumulated
    matmuls, stationary = 16-wide activation slices); fc1 bias is added as a
    broadcast row block, relu happens post-transpose fused into the
    PSUM->SBUF moves.
  * fc2 is computed transposed (stationary = hidden activations) so logits
    land directly as [batch, cls]; its bias rides an extra ones-column of ht
    through the transpose.
  * log_softmax without max-subtraction (logits are bounded); ln(sum) via
    the float-exponent bitcast approximation (err < 0.03 nats, tolerance is
    2e-2 relative), which avoids the 1.3us Ln ACT-table load entirely.
"""

import numpy as np

import concourse.bass as bass
import concourse.tile as tile
from concourse import bacc, mybir
from concourse.bass_utils import run_bass_kernel_spmd

# The axon NTFF profile hook normally lives in antenv.axon_hooks, which this
# image lacks.  Shim it from the boot module's ctypes implementation so
# BASS_TRACE=1 profiling works; degrade silently if unavailable.
try:
    import antenv.axon_hooks  # noqa: F401
except ImportError:
    try:
        import sys as _sys
        import types as _types

        from trn_agent_boot.trn_boot import _ntff_profile_via_ctypes

        _hook = _ntff_profile_via_ctypes('/opt/axon/libaxon_pjrt.so')
        _mod = _types.ModuleType('antenv.axon_hooks')
        _mod.get_axon_ntff_profile_hook = lambda: _hook
        _mod.set_axon_ntff_profile_hook = lambda h: None
        _sys.modules['antenv.axon_hooks'] = _mod
    except Exception:
        pass

F32 = mybir.dt.float32
F16 = mybir.dt.float16
I32 = mybir.dt.int32
AF = mybir.ActivationFunctionType
ALU = mybir.AluOpType
N_NEURONS = 256
N_CORES = 8
B_TOTAL = 128
B = B_TOTAL // N_CORES  # 16 images per core
HW = 28
FC_HID = 200
N_CLS = 10
W = 36  # slot-stack y-columns per group

# float-exponent log: ln(x) ~ ln2 * (bits(x) * 2^-23 - 126.94269504089732)
LOG_SCALE = -float(np.log(2.0)) / (1 << 23)
LOG_BIAS = 126.94269504089732 * float(np.log(2.0))

LAST_RESULT = None  # BassKernelResults of the most recent run (for profiling)


# ---------------------------------------------------------------- schedule
def _schedule(src, tgt):
    n = N_NEURONS
    in_lists = [src[np.where(tgt == i)[0]].astype(np.int64).tolist() for i in range(n)]
    waves = []
    processed = np.zeros(n, bool)
    frontier = [0]
    while True:
        waves.append(list(frontier))
        processed[frontier] = True
        if processed[n - 1]:
            break
        nxt = set()
        for v in frontier:
            for m in tgt[src == v]:
                if not processed[m]:
                    nxt.add(int(m))
        frontier = sorted(nxt)
        assert frontier, "last neuron unreachable"
    return in_lists, waves


def _cone(src, tgt):
    """Returns (steps, fc_live).

    steps: ordered list of (node, [(srckey, channel), ...]) where srckey is
      'x' for the image input or an int neuron id computed in an earlier step.
    fc_live: [(channel_of_255, src_node), ...] live channels of the readout.
    """
    n = N_NEURONS
    in_lists, waves = _schedule(src, tgt)
    wave_of = {}
    for wi, w in enumerate(waves):
        for v in w:
            if v not in wave_of:
                wave_of[v] = wi
    BIG = 1 << 30
    w255 = wave_of[n - 1]
    fc_live = [(c, int(s)) for c, s in enumerate(in_lists[n - 1])
               if wave_of.get(int(s), BIG) < w255]

    live = {}
    stack = [s for _, s in fc_live]
    seen = set()
    while stack:
        v = stack.pop()
        if v in seen:
            continue
        seen.add(v)
        if v == 0:
            live[0] = [('x', 0)]
            continue
        chans = [(int(s), c) for c, s in enumerate(in_lists[v])
                 if wave_of.get(int(s), BIG) < wave_of[v]]
        assert chans, f"cone node {v} has no live channels"
        live[v] = [(s, c) for s, c in chans]
        stack += [s for s, _ in chans]

    steps = sorted(live.items(), key=lambda kv: wave_of[kv[0]])
    return steps, fc_live


# ---------------------------------------------------------- host-side packing
def _toeplitz(w):
    """w [5,5] -> [160, 28] banded matrix over K=(dy, row).

    Slot row r of each 32-row group holds padded-image column (r+2) mod 32,
    so the activation value at x lands at row x (32-aligned writes; wrapped
    rows 28..31 hold the zero x-padding)."""
    T = np.zeros((160, HW), np.float32)
    for dy in range(5):
        for dx in range(5):
            for xc in range(HW):
                T[dy * 32 + (xc + dx - 2) % 32, xc] = w[dy, dx]
    return T


def _xstack(xb):
    """xb [B,28,28] -> [128, W*B] fp16 slot stack, free layout (y, b).

    Group dy (partitions 32dy..32dy+31) holds image row y in column
    (6-dy+y); slot row r within a group carries image x-column (r+2)%32.
    The extra width lets the dy=4 tap read group dy=0 at a +2 window
    (base partition 0, as the ISA requires matching operand bases)."""
    imgx = np.zeros((32, HW, B), np.float32)  # [xcol(padded), y, b]
    imgx[:HW] = xb.transpose(2, 1, 0)
    base = imgx[(np.arange(32) + 2) % 32]     # [slot row, y, b]
    st = np.zeros((4, 32, W, B), np.float32)
    for dy in range(4):
        st[dy, :, 6 - dy:34 - dy, :] = base
    return st.reshape(128, W * B).astype(np.float16)


class _Layout:
    def __init__(self):
        self.n = 0

    def alloc(self, w):
        c0 = self.n
        self.n += w
        return c0


def _pack(steps, fc_live, conv_w, conv_b, fc1_w, fc1_b, fc2_w, fc2_b):
    """Builds consts (f32), mainh toeplitz block (fp16), f1w (fp16)."""
    slots = {}
    lay32 = _Layout()
    lay16 = _Layout()
    for v, chans in steps:
        for j, _ in enumerate(chans):
            slots[('toepA', v, j)] = lay16.alloc(HW)
            slots[('toepB', v, j)] = lay16.alloc(HW)
        slots[('cb', v)] = lay32.alloc(1)
    slots['ident'] = lay32.alloc(B)
    slots['fc2wA'] = lay32.alloc(N_CLS)
    slots['fc2wB'] = lay32.alloc(N_CLS)
    slots['fc1bT'] = lay32.alloc(FC_HID)

    C = np.zeros((128, lay32.n), np.float32)
    TH = np.zeros((128, lay16.n), np.float16)
    for v, chans in steps:
        for j, (skey, ch) in enumerate(chans):
            T = _toeplitz(conv_w[v, 0, ch])
            TH[:, slots[('toepA', v, j)]:slots[('toepA', v, j)] + HW] = T[:128]
            TH[:32, slots[('toepB', v, j)]:slots[('toepB', v, j)] + HW] = T[128:]
        C[:HW, slots[('cb', v)]] = conv_b[v]
    C[:B, slots['ident']:slots['ident'] + B] = np.eye(B, dtype=np.float32)
    w2t = fc2_w.T  # [200, 10]
    C[:128, slots['fc2wA']:slots['fc2wA'] + N_CLS] = w2t[:128]
    C[:FC_HID - 128, slots['fc2wB']:slots['fc2wB'] + N_CLS] = w2t[128:]
    C[FC_HID - 128, slots['fc2wB']:slots['fc2wB'] + N_CLS] = fc2_b  # ht ones col
    C[:B, slots['fc1bT']:slots['fc1bT'] + FC_HID] = fc1_b[None, :]

    n_fc = len(fc_live)
    f1p = np.zeros((128, 1400 * n_fc), np.float16)
    for k, (c, s) in enumerate(fc_live):
        blk = fc1_w[:, c * 784:(c + 1) * 784].reshape(FC_HID, HW, HW)  # [h, y, x]
        arr = blk.reshape(FC_HID, 4, 7, HW).transpose(1, 3, 2, 0)  # [yg, x, ysub, h]
        f1p[:, k * 1400:(k + 1) * 1400] = np.pad(
            arr, ((0, 0), (0, 4), (0, 0), (0, 0))).reshape(128, 1400)
    return C, TH, f1p, slots


# ---------------------------------------------------------- device program
def _build(steps, fc_live, ncols32, ncols16, nfc):
    nc = bacc.Bacc("TRN2", target_bir_lowering=False)
    consts_d = nc.dram_tensor("consts", [128, ncols32], F32, kind="ExternalInput")
    mainh_d = nc.dram_tensor("mainh", [128, ncols16], F16, kind="ExternalInput")
    f1w_d = nc.dram_tensor("f1w", [128, 1400 * nfc], F16, kind="ExternalInput")
    xin_d = nc.dram_tensor("xin", [128, W * B], F16, kind="ExternalInput")
    out_d = nc.dram_tensor("out", [B, N_CLS], F32, kind="ExternalOutput")

    feeds_conv = set()
    for v, chans in steps:
        for skey, _ in chans:
            if skey != 'x':
                feeds_conv.add(skey)
    fc_srcs = [s for _, s in fc_live]
    SL = _SLOTS

    with tile.TileContext(nc) as tc:
        with (
            tc.tile_pool(name="persist", bufs=1) as pool,
            tc.tile_pool(name="cpsum", bufs=1, space="PSUM") as cpp,
            tc.tile_pool(name="fpsum", bufs=1, space="PSUM") as fpp,
        ):
            consts = pool.tile([128, ncols32], F32, tag="consts")
            mainh = pool.tile([128, ncols16], F16, tag="mainh")
            f1w = pool.tile([128, 1400 * nfc], F16, tag="f1w")

            # preload the Exp activation table during the input-DMA shadow
            # (the tail's exp would otherwise eat a 1.3us table load)
            swu = pool.tile([1, 2], F32, tag="swu")
            nc.scalar.memzero(swu[:])
            nc.scalar.activation(swu[:, 0:1], swu[:, 0:1], AF.Exp)

            # activation slot stacks: x comes pre-replicated from the host;
            # conv producers get zeroed tiles filled by relu + Vector copies
            stacks = {'x': pool.tile([128, W * B], F16, name="stA_x", tag="stA_x")}
            for key in sorted(feeds_conv):
                a = pool.tile([128, W * B], F16, name=f"stA_{key}", tag=f"stA_{key}")
                nc.vector.memset(a[:], 0.0)
                stacks[key] = a
            fcstacks = {}
            for sv in set(fc_srcs):
                t = pool.tile([128, 7 * B], F16, name=f"fcst_{sv}", tag=f"fcst_{sv}")
                nc.scalar.memzero(t[:])
                fcstacks[sv] = t
            # ht's extra ones column rides through the transpose into h2's
            # last row and carries the fc2 bias through the fc2 matmul
            ht = pool.tile([B, FC_HID + 1], F32, tag="ht")
            nc.vector.memset(ht[:, FC_HID:FC_HID + 1], 1.0)

            # input DMAs; per-ring FIFO completion means consumers of early
            # DMAs never wait on later ones, so big late loads ride behind
            nc.sync.dma_start(mainh[:], mainh_d[:])
            nc.sync.dma_start(stacks['x'][:], xin_d[:])
            fh = (1400 * nfc) // 2
            nc.scalar.dma_start(consts[:], consts_d[:])
            nc.scalar.dma_start(f1w[:, :fh], f1w_d[:, :fh])
            nc.gpsimd.dma_start(f1w[:, fh:], f1w_d[:, fh:])

            # --- conv chain ---
            for v, chans in steps:
                cb0 = SL[('cb', v)]
                bias = consts[:HW, cb0:cb0 + 1]
                nch = len(chans)

                ps = cpp.tile([HW, HW * B], F32, tag="convps")
                for j, (skey, ch) in enumerate(chans):
                    a0 = SL[('toepA', v, j)]
                    b0 = SL[('toepB', v, j)]
                    a = stacks[skey]
                    # dy=4 tap first: reads the dy0 group (base partition 0,
                    # matching the stationary) at a +2 column window, so it
                    # only needs the first Vector copy
                    nc.tensor.matmul(ps[:], mainh[:32, b0:b0 + HW],
                                     a[0:32, 8 * B:(8 + HW) * B],
                                     start=(j == 0), stop=False)
                    nc.tensor.matmul(ps[:], mainh[:, a0:a0 + HW],
                                     a[:, 4 * B:(4 + HW) * B],
                                     start=False, stop=(j == nch - 1))

                if v in feeds_conv:
                    # relu+bias lands in the dy=2 slot group (one Scalar
                    # PSUM read); Vector replicates to dy3, dy1, dy0
                    a = stacks[v]
                    g2 = a[64:64 + HW, 4 * B:(4 + HW) * B]
                    nc.scalar.activation(g2, ps[:], AF.Relu, bias=bias, scale=1.0)
                    for dy in (0, 3, 1):
                        nc.vector.tensor_copy(
                            a[dy * 32:dy * 32 + HW, (6 - dy) * B:(34 - dy) * B], g2)
                if v in fcstacks:
                    # reshape y into (ygroup, ysub) partition quarters; the
                    # writes column-overlap so they serialize regardless of
                    # engine -- keep them all on Scalar (fewest semaphores)
                    fst = fcstacks[v]
                    for g in range(4):
                        nc.scalar.activation(fst[g * 32:g * 32 + HW, :],
                                             ps[:, 7 * g * B:(7 * g + 7) * B],
                                             AF.Relu, bias=bias, scale=1.0)

            # --- fc1: activations stationary, hidden units streamed ---
            p1 = fpp.tile([B, FC_HID], F32, tag="p1")
            for k in range(nfc):
                fst = fcstacks[fc_live[k][1]]
                for sj in range(7):
                    i = k * 7 + sj
                    nc.tensor.matmul(p1[:], fst[:, sj * B:(sj + 1) * B],
                                     f1w[:, i * 200:(i + 1) * 200],
                                     start=(i == 0), stop=(i == 7 * nfc - 1))
            # add fc1 bias (broadcast rows in consts); relu happens after the
            # transpose, fused into the PSUM->SBUF moves
            nc.vector.tensor_add(ht[:, 0:FC_HID], p1[:],
                                 consts[:B, SL['fc1bT']:SL['fc1bT'] + FC_HID])
            idn = consts[:B, SL['ident']:SL['ident'] + B]
            t1 = fpp.tile([128, B], F32, tag="tt", bufs=2)
            t2 = fpp.tile([FC_HID - 128 + 1, B], F32, tag="tt", bufs=2)
            nc.tensor.transpose(t1[:], ht[:, 0:128], idn)
            nc.tensor.transpose(t2[:], ht[:, 128:FC_HID + 1], idn)
            h1 = pool.tile([128, B], F32, tag="h1")
            h2 = pool.tile([FC_HID - 128 + 1, B], F32, tag="h2")
            nc.scalar.activation(h1[:], t1[:], AF.Relu, bias=0.0, scale=1.0)
            nc.vector.tensor_scalar_max(h2[:], t2[:], 0.0)

            # --- fc2 transposed (stationary = hidden) + log_softmax ---
            pst = fpp.tile([B, N_CLS], F32, tag="l2", bufs=1)
            nc.tensor.matmul(pst[:], h1[:],
                             consts[:128, SL['fc2wA']:SL['fc2wA'] + N_CLS],
                             start=True, stop=False)
            nc.tensor.matmul(pst[:], h2[:],
                             consts[:FC_HID - 128 + 1, SL['fc2wB']:SL['fc2wB'] + N_CLS],
                             start=False, stop=True)
            # log_softmax without max-subtraction (logits bounded, fp32 exp
            # safe); lse via float-exponent bitcast, so no Ln table load
            ex = pool.tile([B, N_CLS], F32, tag="ex")
            sm = pool.tile([B, 1], F32, tag="sm")
            nc.scalar.activation(ex[:], pst[:], AF.Exp, bias=0.0, scale=1.0,
                                 accum_out=sm[:])
            kf = pool.tile([B, 1], F32, tag="kf")
            nc.vector.tensor_copy(kf[:], sm[:].bitcast(I32))
            nls = pool.tile([B, 1], F32, tag="nls")
            nc.vector.tensor_scalar(nls[:], kf[:], LOG_SCALE, LOG_BIAS,
                                    ALU.mult, ALU.add)
            res = pool.tile([B, N_CLS], F32, tag="res")
            nc.scalar.activation(res[:], pst[:], AF.Identity, bias=nls[:], scale=1.0)
            nc.sync.dma_start(out_d[:], res[:])
    nc.compile()
    return nc


_SLOTS = None
_PROG_CACHE = {}


def kernel(x, src, tgt, conv_w, conv_b, fc1_w, fc1_b, fc2_w, fc2_b):
    global _SLOTS, LAST_RESULT
    x = np.asarray(x, np.float32)
    src = np.asarray(src, np.int32)
    tgt = np.asarray(tgt, np.int32)
    conv_w = np.asarray(conv_w, np.float32)
    conv_b = np.asarray(conv_b, np.float32)
    fc1_w = np.asarray(fc1_w, np.float32)
    fc1_b = np.asarray(fc1_b, np.float32)
    fc2_w = np.asarray(fc2_w, np.float32)
    fc2_b = np.asarray(fc2_b, np.float32)

    steps, fc_live = _cone(src, tgt)
    C, TH, f1p, slots = _pack(steps, fc_live, conv_w, conv_b,
                              fc1_w, fc1_b, fc2_w, fc2_b)
    _SLOTS = slots

    key = (tuple((v, tuple(ch)) for v, ch in steps), tuple(fc_live),
           C.shape[1], TH.shape[1])
    if key not in _PROG_CACHE:
        _PROG_CACHE[key] = _build(steps, fc_live, C.shape[1], TH.shape[1],
                                  len(fc_live))
    nc = _PROG_CACHE[key]

    xs = x[:, 0]  # [128, 28, 28]
    in_maps = []
    for c in range(N_CORES):
        in_maps.append({"consts": C, "mainh": TH, "f1w": f1p,
                        "xin": _xstack(xs[c * B:(c + 1) * B])})

    LAST_RESULT = run_bass_kernel_spmd(nc, in_maps, list(range(N_CORES)))
    out = np.concatenate([r["out"] for r in LAST_RESULT.results], axis=0)
    return out.astype(np.float32)
